# revision 22
# baseline (speedup 1.0000x reference)
"""Distributed Trainium2 Bass kernel for nn_AddModelWithAttentionStacked.

Sharding: mesh B(4) x L(2) over 8 NeuronCores. Core c owns batch b=c//2 and
sequence rows [r0, r0+256) with r0 = (c%2)*256. Activations are kept
feature-major (E on partitions) in SBUF. Per layer the pair all-gathers the
updated xsa shard (keys/values for attention); the loss head is
vocab-parallel over all 8 cores (per-shard sum-exp + AllGather of partials).

Perf structure vs the original baseline:
- fp8(e4m3) DoubleRow matmuls everywhere heavy (2x PE throughput), with
  static power-of-two scales chosen from measured magnitudes.
- transposed attention scores (queries on partitions): softmax sums on the
  free axis, normalization folded into the PE transpose via diag(1/sum).
- sequence rolls as shifted DVE copies (+ masked boundary blend) instead of
  shift matmuls.
- layer norm stats kept partition-major (row index on partitions) so the
  whole stats chain is ~60ns DVE ops instead of 2us 1-partition ops.
- loss-head sum-exp combined via AllGather + local reduce (cheaper floor
  than AllReduce), fp8 logits, all embed tiles persistent in SBUF.
"""

import numpy as np
import ml_dtypes

G, E, K, D, B, L, M, KN = 32000, 256, 8, 6, 4, 512, 64, 4
STEP, EPS = 0.05, 1.0
NCORES = 8
RL = L // 2          # 256 local rows
VS = G // NCORES     # 4000 vocab shard
VC = 500             # vocab chunk
NVC = VS // VC       # 8

_D_EFF = D
_DEBUG = False
_TRACE = False
_CACHE = {}

f8np = ml_dtypes.float8_e4m3
bf16np = ml_dtypes.bfloat16

# static scales (power of two), from measured magnitudes:
#   xsa max 0.31, q max 0.39, a1/a2 max 0.32, yik max 0.013, actb max 0.027,
#   weights max 0.28, xx/xx2/xx3/xxw max 0.53, probs ~= 1/512.
SA = 256.0     # activations: loc/full/nat/rp1/rm1/a1/a2/qT/lptok/xx*/embt-in
SW = 512.0     # weights: WTC/WQT/WDT/WKCT/WEM/ETT/EMBT
SWO = 32.0     # Wo
SXID = 8192.0  # yik
SACT = 4096.0  # relu(Wd @ xid)
SEST = 32768.0  # softmax probs


def _f8(x, s):
    return np.ascontiguousarray(
        np.clip(np.asarray(x, np.float32) * s, -240.0, 240.0).astype(f8np))


def _bf(x):
    return np.ascontiguousarray(np.asarray(x, np.float32).astype(bf16np))


def _f32(x):
    return np.ascontiguousarray(np.asarray(x, np.float32))


def _norm_np(x):
    return x / (EPS + np.std(x, axis=-1, ddof=1, keepdims=True))


def _prep(inputs):
    masked = np.asarray(inputs['masked'])
    unmasked = np.asarray(inputs['unmasked'])
    mask = np.asarray(inputs['mask'])
    summer = np.asarray(inputs['summer'], np.float32)
    embed = np.asarray(inputs['embed'], np.float32)
    pos = np.asarray(inputs['pos'], np.float32)
    Wt = np.asarray(inputs['Wt'], np.float32)
    Wc = np.asarray(inputs['Wc'], np.float32)
    Wq = np.asarray(inputs['Wq'], np.float32)
    Wd = np.asarray(inputs['Wd'], np.float32)
    Wo = np.asarray(inputs['Wo'], np.float32)
    Wkc = np.asarray(inputs['Wkc'], np.float32)
    bkc = np.asarray(inputs['bkc'], np.float32)
    Wem = np.asarray(inputs['Wem'], np.float32)

    # ---- shared (identical on all cores) ----
    # WTC: [d, p, mat(4), kc(2), mc(2), c(128)] -> flat (D, 128, 2048)
    def blk_nat(w):  # [d, p, kc, mc, c] = w[d, kc*128+p, mc*128+c]
        return w.reshape(D, 2, 128, 2, 128).transpose(0, 2, 1, 3, 4)

    def blk_tr(w):   # [d, p, kc, mc, c] = w[d, mc*128+c, kc*128+p]
        return w.reshape(D, 2, 128, 2, 128).transpose(0, 4, 3, 1, 2)

    wtc = np.stack([blk_nat(Wt), blk_nat(Wc), blk_tr(Wc), blk_tr(Wt)], axis=2)
    WTC = _f8(wtc.reshape(D, 128, 4 * 2 * 2 * 128), SW)

    # WQT: [d, p, kc(2), mc(16), c] = Wq[d, mc*128+c, kc*128+p]
    wq = Wq.reshape(D, 16, 128, 2, 128).transpose(0, 4, 3, 1, 2)
    WQT = _f8(wq.reshape(D, 128, 2 * 16 * 128), SW)

    # WDT: [d, kcp(8), p, i(2), mc(16), c] = Wd[d, mc*128+c, (2kcp+i)*128+p]
    wd = Wd.reshape(D, 16, 128, 16, 128).transpose(0, 3, 4, 1, 2)
    wd = wd.reshape(D, 8, 2, 128, 16, 128).transpose(0, 1, 3, 2, 4, 5)
    WDT = _f8(wd.reshape(D, 8, 128, 2 * 16 * 128), SW)

    # WO: [d, kcp(8), p, i(2), mc(2), c] = Wo[d, (2kcp+i)*128+p, mc*128+c]
    wo = Wo.reshape(D, 8, 2, 128, 2, 128).transpose(0, 1, 3, 2, 4, 5)
    WO = _f8(wo.reshape(D, 8, 128, 2 * 2 * 128), SWO)

    # WKCT: [p, fc(2), knec(8), c] = Wkc[knec*128+c, fc*128+p]
    wk = Wkc.reshape(8, 128, 2, 128).transpose(3, 2, 0, 1)
    WKCT = _f8(wk.reshape(128, 2 * 8 * 128), SW)

    # WEM: [p, kc(2), ec(2), c] = Wem[kc*128+p, ec*128+c]
    we = Wem.reshape(2, 128, 2, 128).transpose(1, 0, 2, 3)
    WEM = _f8(we.reshape(128, 2 * 2 * 128), SW)

    BKC = _f32(bkc.reshape(8, 128).T * (SW * SA))  # (128, 8) [p, knec]

    # ---- derived host math ----
    xsa0 = _norm_np(embed[masked] + pos[None])  # (B, L, E) f32
    tgt = np.take_along_axis(unmasked, mask, axis=1)  # (B, M)
    # ETT: [p, ec(2), n(1024)] = embed[tgt[b, j], ec*128+p], n = b*256+j*4+kn
    tgt_rep = np.repeat(tgt.reshape(B * M), KN)
    ett = embed[tgt_rep]  # (1024, 256)
    ETT = _f8(ett.reshape(1024, 2, 128).transpose(2, 1, 0).reshape(128, 2048),
              SW)

    # WIND: [i, t(2), bcol(4)]: r = t*128+i = b*64+m -> summer[b, m]*(bcol==b)
    wind = np.zeros((128, 2, B), np.float32)
    r = np.arange(256)
    bi, mi = r // M, r % M
    wind[r % 128, r // 128, bi] = summer[bi, mi]
    WIND = _f32(wind.reshape(128, 2 * B))

    shared = dict(WTC=WTC, WQT=WQT, WDT=WDT, WO=WO, WKCT=WKCT, WEM=WEM,
                  BKC=BKC, ETT=ETT, WIND=WIND)

    # ---- per-core ----
    in_maps = []
    for c in range(NCORES):
        b, h = c // 2, c % 2
        r0 = h * RL
        m = dict(shared)
        # XSA0: (128, 2*256) f32: [p, ec*256+j] = xsa0[b, r0+j, ec*128+p]
        x0 = xsa0[b, r0:r0 + RL]  # (256, 256)
        m['XSA0'] = _f32(x0.reshape(RL, 2, 128).transpose(2, 1, 0)
                         .reshape(128, 512))
        # FULL0: whole pair batch, feature-major: [p, ec*512+g]
        m['FULL0'] = _bf(xsa0[b].reshape(L, 2, 128).transpose(2, 1, 0)
                         .reshape(128, 1024))
        # SC0 partition-major: (128, 4) [jp, (sx0, sx1, qx0, qx1)]
        su = x0.sum(-1).reshape(2, 128)      # [jb, jp]
        qu = (x0 * x0).sum(-1).reshape(2, 128)
        m['SC0'] = _f32(np.stack([su[0], su[1], qu[0], qu[1]], axis=1))
        # MSEL: [p, lb(4), j(64)] = 1 iff lb*128+p == mask[b, j]   (fp8)
        ms = np.zeros((L, M), np.float32)
        ms[mask[b], np.arange(M)] = 1.0
        m['MSEL'] = _f8(ms.reshape(4, 128, M).transpose(1, 0, 2)
                        .reshape(128, 4 * M), 1.0)
        # PM: (128, 2) f32: col0 = 1 if partner is rank 0 of pair (h==1)
        pm = np.zeros((128, 2), np.float32)
        pm[:, 0] = SA if h == 1 else 0.0
        pm[:, 1] = SA if h == 0 else 0.0
        m['PM'] = _f32(pm)
        # EMBT: [vc(8), p, ec(2), n(500)] = embed[c*4000+vc*500+n, ec*128+p]
        shard = embed[c * VS:(c + 1) * VS]  # (4000, 256)
        et = shard.reshape(NVC, VC, 2, 128).transpose(0, 3, 2, 1)
        m['EMBT'] = _f8(et.reshape(NVC, 128, 2 * VC), SW)
        in_maps.append(m)

    aux = dict(summer=summer)
    return in_maps, aux


def _build(d_eff, debug):
    import concourse.bass as bass
    import concourse.tile as tile
    from concourse import mybir, bacc
    from concourse.masks import make_identity
    from contextlib import ExitStack

    dt = mybir.dt
    AF = mybir.ActivationFunctionType
    AX = mybir.AxisListType
    STT = mybir.AluOpType
    DR = mybir.MatmulPerfMode.DoubleRow

    nc = bacc.Bacc("TRN2", num_devices=NCORES)

    def par(name, shape, dtype=dt.float8e4):
        return nc.dram_tensor(name, shape, dtype, kind="ExternalInput")

    P = {}
    P['WTC'] = par('WTC', [D, 128, 2048])
    P['WQT'] = par('WQT', [D, 128, 4096])
    P['WDT'] = par('WDT', [D, 8, 128, 4096])
    P['WO'] = par('WO', [D, 8, 128, 512])
    P['WKCT'] = par('WKCT', [128, 2048])
    P['WEM'] = par('WEM', [128, 512])
    P['BKC'] = par('BKC', [128, 8], dt.float32)
    P['ETT'] = par('ETT', [128, 2048])
    P['WIND'] = par('WIND', [128, 8], dt.float32)
    P['XSA0'] = par('XSA0', [128, 512], dt.float32)
    P['FULL0'] = par('FULL0', [128, 1024], dt.bfloat16)
    P['SC0'] = par('SC0', [128, 4], dt.float32)
    P['MSEL'] = par('MSEL', [128, 256])
    P['PM'] = par('PM', [128, 2], dt.float32)
    P['EMBT'] = par('EMBT', [NVC, 128, 1000])

    out_t = nc.dram_tensor("out", [4, 1], dt.float32, kind="ExternalOutput")
    dbg = {}

    def dbg_out(name, shape, dtype):
        if debug and name not in dbg:
            dbg[name] = nc.dram_tensor(name, shape, dtype,
                                       kind="ExternalOutput")
        return dbg.get(name)

    with tile.TileContext(nc) as tc, ExitStack() as ctx:
        con = ctx.enter_context(tc.tile_pool(name="con", bufs=1))
        pers = ctx.enter_context(tc.tile_pool(name="pers", bufs=1))
        sb = ctx.enter_context(tc.tile_pool(name="sb", bufs=2))
        mpool = ctx.enter_context(tc.tile_pool(name="mpool", bufs=2))
        wdp = ctx.enter_context(tc.tile_pool(name="wdp", bufs=24))
        rows = ctx.enter_context(tc.tile_pool(name="rows", bufs=1))
        hp = ctx.enter_context(tc.tile_pool(name="hp", bufs=1))
        ppx = ctx.enter_context(tc.tile_pool(name="ppx", bufs=1, space="PSUM"))
        pst = ctx.enter_context(tc.tile_pool(name="pst", bufs=2, space="PSUM"))
        ptp = ctx.enter_context(tc.tile_pool(name="ptp", bufs=1, space="PSUM"))
        ppq = ctx.enter_context(tc.tile_pool(name="ppq", bufs=1, space="PSUM"))
        ppy = ctx.enter_context(tc.tile_pool(name="ppy", bufs=2, space="PSUM"))
        pps = ctx.enter_context(tc.tile_pool(name="pps", bufs=1, space="PSUM"))
        dram = ctx.enter_context(tc.tile_pool(name="dram", bufs=2,
                                              space="DRAM"))

        mm = nc.tensor.matmul
        act = nc.scalar.activation
        V = nc.vector

        # rendezvous: tiny all-reduce so core-start skew is absorbed here
        rdv_in = dram.tile([128], dt.float32, tag='rdv_in')
        rdv_out = dram.tile([128], dt.float32, tag='rdv_out')
        rdv_sb = con.tile([1, 128], dt.float32)
        V.memset(rdv_sb, 0.0)
        nc.gpsimd.dma_start(out=rdv_in[:], in_=rdv_sb[:])
        nc.gpsimd.collective_compute(
            "AllReduce", mybir.AluOpType.add,
            replica_groups=[list(range(NCORES))],
            ins=[rdv_in.opt()], outs=[rdv_out.opt()],
        )

        # master xsa (feature-major f32); layer-0 gather comes from the host
        master = mpool.tile([128, 512], dt.float32, tag='master')
        nc.sync.dma_start(out=master[:], in_=P['XSA0'][:])
        loc_bf = mpool.tile([128, 512], dt.bfloat16, tag='locb')
        act(out=loc_bf[:], in_=master[:], func=AF.Copy)
        loc = mpool.tile([128, 512], dt.float8e4, tag='loc')
        V.tensor_scalar_mul(out=loc[:], in0=master[:], scalar1=SA)

        def comm_gather(loc_t):
            """AllGather pair's xsa (bf16) -> full_bf (128, 2*512) + fp8 copy."""
            ag_in = dram.tile([2, 128, 256], dt.bfloat16, tag='ag_in')
            ag_out = dram.tile([4, 128, 256], dt.bfloat16, tag='ag_out')
            for ec in range(2):
                nc.gpsimd.dma_start(out=ag_in[ec],
                                    in_=loc_t[:, ec * 256:(ec + 1) * 256])
            nc.gpsimd.collective_compute(
                "AllGather", mybir.AluOpType.bypass,
                replica_groups=[[0, 1], [2, 3], [4, 5], [6, 7]],
                ins=[ag_in.opt()], outs=[ag_out.opt()],
            )
            full_bf = sb.tile([128, 1024], dt.bfloat16, tag='fullb',
                              name='full_bf')
            for r in range(2):
                for ec in range(2):
                    nc.gpsimd.dma_start(
                        out=full_bf[:, ec * 512 + r * 256: ec * 512 + r * 256 + 256],
                        in_=ag_out[r * 2 + ec])
            full = sb.tile([128, 1024], dt.float8e4, tag='full', name='full')
            V.tensor_scalar_mul(out=full[:, 0:512], in0=full_bf[:, 0:512],
                                scalar1=SA)
            V.tensor_scalar_mul(out=full[:, 512:1024],
                                in0=full_bf[:, 512:1024], scalar1=SA)
            return full_bf, full

        full_bf0 = sb.tile([128, 1024], dt.bfloat16, tag='fullb',
                           name='full_bf')
        nc.sync.dma_start(out=full_bf0[:], in_=P['FULL0'][:])
        full8_0 = sb.tile([128, 1024], dt.float8e4, tag='full', name='full')
        V.tensor_scalar_mul(out=full8_0[:, 0:512], in0=full_bf0[:, 0:512],
                            scalar1=SA)
        V.tensor_scalar_mul(out=full8_0[:, 512:1024],
                            in0=full_bf0[:, 512:1024], scalar1=SA)
        full0 = (full_bf0, full8_0)

        # constants
        ident = con.tile([128, 128], dt.bfloat16)
        make_identity(nc, ident)
        ident8 = con.tile([128, 128], dt.float8e4)
        V.tensor_copy(out=ident8[:], in_=ident[:])
        ones128 = con.tile([128, 128], dt.bfloat16)
        V.memset(ones128, 1.0)
        ones_cb = con.tile([128, 1], dt.bfloat16)
        V.memset(ones_cb, 1.0)

        # persistent inputs (loaded once; EMBT persistent for both passes)
        msel = pers.tile([128, 256], dt.float8e4)
        nc.gpsimd.dma_start(out=msel[:], in_=P['MSEL'][:])
        wkct = pers.tile([128, 2048], dt.float8e4)
        nc.gpsimd.dma_start(out=wkct[:], in_=P['WKCT'][:])
        wem = pers.tile([128, 512], dt.float8e4)
        nc.gpsimd.dma_start(out=wem[:], in_=P['WEM'][:])
        bkc_sb = pers.tile([128, 8], dt.float32)
        nc.gpsimd.dma_start(out=bkc_sb[:], in_=P['BKC'][:])
        ett = pers.tile([128, 2048], dt.float8e4)
        nc.gpsimd.dma_start(out=ett[:], in_=P['ETT'][:])
        wind = pers.tile([128, 8], dt.float32)
        nc.gpsimd.dma_start(out=wind[:], in_=P['WIND'][:])
        pmask = pers.tile([128, 2], dt.float32)
        nc.gpsimd.dma_start(out=pmask[:], in_=P['PM'][:])
        embt = [pers.tile([128, 1000], dt.float8e4, name=f'embt{vc}')
                for vc in range(NVC)]
        for vc in range(NVC):
            nc.gpsimd.dma_start(out=embt[vc][:], in_=P['EMBT'][vc])
        st_carry = mpool.tile([128, 4], dt.float32, tag='stc')
        nc.sync.dma_start(out=st_carry[:], in_=P['SC0'][:])

        def nat_transpose(full_bf):
            """nat (128, 1024) fp8 xSA: [keys_p, kb*256 + ec*128 + c]."""
            nat = sb.tile([128, 1024], dt.float8e4, tag='nat', name='nat')
            nat_4d = nat[:].rearrange('p (kb e c) -> p kb e c', kb=4, e=2)
            for ec in range(2):
                tp = ptp.tile([128, 512], dt.bfloat16, tag='tp', bufs=1,
                              name='tp')
                for kb in range(4):
                    nc.tensor.transpose(
                        tp[:, kb * 128:(kb + 1) * 128],
                        full_bf[:, ec * 512 + kb * 128: ec * 512 + kb * 128 + 128],
                        ident[:])
                if ec == 0:
                    V.tensor_scalar_mul(
                        out=nat_4d[:, :, ec, :],
                        in0=tp[:].rearrange('p (kb c) -> p kb c', kb=4),
                        scalar1=SA)
                else:
                    act(out=nat_4d[:, :, ec, :],
                        in_=tp[:].rearrange('p (kb c) -> p kb c', kb=4),
                        func=AF.Copy, scale=SA)
            return nat

        for d in range(d_eff):
            # --- A: pair all-gather of xsa (issued first; qT overlaps it) ---
            full_bf, full = full0 if d == 0 else comm_gather(loc_bf)
            full_r = full[:].rearrange('p (e g) -> p e g', e=2)

            # --- B: qT projection (local only; overlaps the collective) ---
            wq = sb.tile([128, 4096], dt.float8e4, tag='wq', bufs=2, name='wq')
            nc.sync.dma_start(out=wq[:], in_=P['WQT'][d])
            wq_r = wq[:].rearrange('p (k x) -> p k x', k=2)
            loc_r = loc[:].rearrange('p (e j) -> p e j', e=2)
            qT = sb.tile([128, 4096], dt.float8e4, tag='qT', bufs=1, name='qT')
            for mh in range(8):
                if mh % 2 == 0:
                    q_ps = ppq.tile([128, 512], dt.float32, tag='qps', bufs=1,
                                    name='q_ps')
                else:
                    q_ps = pst.tile([128, 512], dt.float32, tag='sT', bufs=2,
                                    name='q_ps2')
                for i in range(2):
                    mc = mh * 2 + i
                    mm(q_ps[:, i * 256:(i + 1) * 256],
                       wq_r[:, :, mc * 128:(mc + 1) * 128], loc_r,
                       start=True, stop=True, perf_mode=DR)
                V.tensor_scalar_mul(out=qT[:, mh * 512:(mh + 1) * 512],
                                    in0=q_ps[:], scalar1=1.0 / 512.0)

            # --- weight loads for this layer (overlap downstream compute) ---
            wtc = sb.tile([128, 2048], dt.float8e4, tag='wtc', bufs=2,
                          name='wtc')
            nc.sync.dma_start(out=wtc[:], in_=P['WTC'][d])
            wdt = []
            for kcp in range(8):
                w = wdp.tile([128, 4096], dt.float8e4, tag='wd',
                             name=f'wd{kcp}')
                nc.sync.dma_start(out=w[:], in_=P['WDT'][d, kcp])
                wdt.append(w)
            wo = []
            for kcp in range(8):
                w = wdp.tile([128, 512], dt.float8e4, tag='wo',
                             name=f'wo{kcp}')
                nc.sync.dma_start(out=w[:], in_=P['WO'][d, kcp])
                wo.append(w)

            # --- C: rolled windows via shifted copies + boundary blend ---
            rolled = {}
            for nm, off in (('p1', -1), ('m1', +1)):
                rt = sb.tile([128, 512], dt.float8e4, tag=f'r{nm}',
                             name=f'r{nm}')
                rt_r = rt[:].rearrange('p (e j) -> p e j', e=2)
                if off == -1:
                    V.tensor_copy(out=rt_r[:, :, 1:256], in_=loc_r[:, :, 0:255])
                    # col 0 = partner row: local j=255 of the other rank
                    cand = (255, 511)   # (partner=rank0, partner=rank1)
                else:
                    V.tensor_copy(out=rt_r[:, :, 0:255], in_=loc_r[:, :, 1:256])
                    cand = (0, 256)     # m1: partner's j=0 -> (rank0: 0, rank1: 256)
                for ec in range(2):
                    c0, c1 = cand
                    ta = rows.tile([128, 1], dt.float32, tag='bta', bufs=4,
                                   name='bta')
                    tb = rows.tile([128, 1], dt.float32, tag='btb', bufs=4,
                                   name='btb')
                    V.tensor_mul(ta[:], full_bf[:, ec * 512 + c0: ec * 512 + c0 + 1],
                                 pmask[:, 0:1])
                    V.tensor_mul(tb[:], full_bf[:, ec * 512 + c1: ec * 512 + c1 + 1],
                                 pmask[:, 1:2])
                    dst = rt_r[:, ec, 0:1] if off == -1 else rt_r[:, ec, 255:256]
                    V.tensor_add(dst, ta[:], tb[:])
                rolled[nm] = rt

            # --- D: nat layout (rows on partitions) ---
            nat = nat_transpose(full_bf)
            nat_r = nat[:].rearrange('p (kb x) -> p kb x', kb=4)

            # --- E: local transition terms accumulated into xsad psum ---
            xsad_ps = ppx.tile([128, 512], dt.float32, tag='xsad',
                               name='xsad_ps')
            wtc_r = wtc[:].rearrange('p (mat k x) -> p mat k x', mat=4, k=2)

            def wtc_pair(mat, mc):
                return wtc_r[:, mat, :, mc * 128:(mc + 1) * 128]

            a1 = sb.tile([128, 512], dt.float8e4, tag='a1', name='a1')
            rp1_r = rolled['p1'][:].rearrange('p (e j) -> p e j', e=2)
            rm1_r = rolled['m1'][:].rearrange('p (e j) -> p e j', e=2)
            for mc in range(2):
                a_ps = ppy.tile([128, 256], dt.float32, tag='aps', name='a_ps')
                mm(a_ps[:], wtc_pair(0, mc), rp1_r, start=True, stop=True,
                   perf_mode=DR)
                act(out=a1[:, mc * 256:(mc + 1) * 256], in_=a_ps[:],
                    func=AF.Relu, scale=1.0 / 512.0)
            a1_r = a1[:].rearrange('p (e j) -> p e j', e=2)
            for mc in range(2):
                mm(xsad_ps[:, mc * 256:(mc + 1) * 256], wtc_pair(1, mc), a1_r,
                   start=True, stop=False, perf_mode=DR)
            a2 = sb.tile([128, 512], dt.float8e4, tag='a2', name='a2')
            for mc in range(2):
                a_ps = ppy.tile([128, 256], dt.float32, tag='aps',
                                name='a_ps2')
                mm(a_ps[:], wtc_pair(2, mc), rm1_r, start=True, stop=True,
                   perf_mode=DR)
                act(out=a2[:, mc * 256:(mc + 1) * 256], in_=a_ps[:],
                    func=AF.Relu, scale=1.0 / 512.0)
            a2_r = a2[:].rearrange('p (e j) -> p e j', e=2)
            for mc in range(2):
                mm(xsad_ps[:, mc * 256:(mc + 1) * 256], wtc_pair(3, mc), a2_r,
                   start=False, stop=False, perf_mode=DR)

            if debug and d == 0:
                t = dbg_out('dbg_rp1', [128, 512], dt.float8e4)
                nc.sync.dma_start(out=t[:], in_=rolled['p1'][:])
                t = dbg_out('dbg_rm1', [128, 512], dt.float8e4)
                nc.sync.dma_start(out=t[:], in_=rolled['m1'][:])
                t = dbg_out('dbg_a1', [128, 512], dt.float8e4)
                nc.sync.dma_start(out=t[:], in_=a1[:])
                t = dbg_out('dbg_q', [128, 4096], dt.float8e4)
                nc.sync.dma_start(out=t[:], in_=qT[:])

            # --- F: attention heads, transposed scores ---
            xid = sb.tile([128, 4096], dt.float8e4, tag='xid', bufs=1,
                          name='xid')
            xid_r = xid[:].rearrange('p (k j) -> p k j', k=16)
            qT_r = qT[:].rearrange('p (h k q) -> p h k q', h=8, k=2)

            def head_front(h):
                estT = sb.tile([128, 1024], dt.bfloat16, tag='estT', bufs=3,
                               name='estT')
                ssum = rows.tile([128, 2], dt.float32, tag='ssum', bufs=3,
                                 name='ssum')
                for qb in range(2):
                    sT = pst.tile([128, 512], dt.float32, tag='sT', bufs=2,
                                  name='sT')
                    mm(sT[:], qT_r[:, h, :, qb * 128:(qb + 1) * 128], full_r,
                       start=True, stop=True, perf_mode=DR)
                    act(out=estT[:, qb * 512:(qb + 1) * 512],
                        in_=sT[:], func=AF.Exp,
                        scale=1.0 / (16.0 * 65536.0))
                    V.reduce_sum(out=ssum[:, qb:qb + 1],
                                 in_=estT[:, qb * 512:(qb + 1) * 512],
                                 axis=AX.X)
                rec = rows.tile([128, 2], dt.float32, tag='rec', bufs=3,
                                name='rec')
                V.reciprocal(rec[:], ssum[:])
                dg = [rows.tile([128, 128], dt.bfloat16, tag=f'dg{qb}',
                                bufs=3, name=f'dg{qb}') for qb in range(2)]
                for qb in range(2):
                    nc.vector.tensor_scalar(
                        out=dg[qb][:], in0=ident[:], scalar1=rec[:, qb:qb + 1],
                        scalar2=SEST, op0=STT.mult, op1=STT.mult)
                return estT, dg

            def head_back(h, estT, dg):
                est = sb.tile([128, 1024], dt.float8e4, tag='est', bufs=3,
                              name='est')
                for kb in range(4):
                    tp = ppy.tile([128, 256], dt.float32, tag='aps',
                                  name='tpe')
                    for qb in range(2):
                        mm(tp[:, qb * 128:(qb + 1) * 128],
                           estT[:, qb * 512 + kb * 128: qb * 512 + kb * 128 + 128],
                           dg[qb][:], start=True, stop=True)
                    dst = est[:, kb * 256:(kb + 1) * 256]
                    if kb % 2 == 0:
                        V.tensor_copy(out=dst, in_=tp[:])
                    else:
                        act(out=dst, in_=tp[:], func=AF.Copy)
                est_r = est[:].rearrange('p (kb q) -> p kb q', kb=4)
                for ec in range(2):
                    y_ps = ppy.tile([128, 256], dt.float32, tag='aps',
                                    name='y_ps')
                    for kbp in range(2):
                        mm(y_ps[:],
                           nat_r[:, 2 * kbp:2 * kbp + 2,
                                 ec * 128:(ec + 1) * 128],
                           est_r[:, 2 * kbp:2 * kbp + 2, :],
                           start=(kbp == 0), stop=(kbp == 1), perf_mode=DR)
                    dst_x = xid[:, (h * 2 + ec) * 256:(h * 2 + ec + 1) * 256]
                    if h % 2 == 0:
                        act(out=dst_x, in_=y_ps[:], func=AF.Copy,
                            scale=1.0 / 1024.0)
                    else:
                        V.tensor_scalar_mul(out=dst_x, in0=y_ps[:],
                                            scalar1=1.0 / 1024.0)

            prev = None
            for h in range(8):
                cur = head_front(h)
                if prev is not None:
                    head_back(h - 1, *prev)
                if debug and d == 0 and h == 0:
                    t = dbg_out('dbg_estT0', [128, 1024], dt.bfloat16)
                    nc.sync.dma_start(out=t[:], in_=cur[0][:])
                prev = cur
            head_back(7, *prev)

            if debug and d == 0:
                t = dbg_out('dbg_xid', [128, 4096], dt.float8e4)
                nc.sync.dma_start(out=t[:], in_=xid[:])

            # --- G: dense relu (Wd), fp8 DoubleRow ---
            actb = sb.tile([128, 4096], dt.float8e4, tag='actb', bufs=1,
                           name='actb')
            for mc in range(16):
                act_ps = ppy.tile([128, 256], dt.float32, tag='aps',
                                  name='act_ps')
                for kcp in range(8):
                    wdr = wdt[kcp][:].rearrange('p (i x) -> p i x', i=2)
                    mm(act_ps[:], wdr[:, :, mc * 128:(mc + 1) * 128],
                       xid_r[:, 2 * kcp:2 * kcp + 2, :],
                       start=(kcp == 0), stop=(kcp == 7), perf_mode=DR)
                if mc % 2 == 0:
                    act(out=actb[:, mc * 256:(mc + 1) * 256], in_=act_ps[:],
                        func=AF.Relu, scale=1.0 / 1024.0)
                else:
                    V.tensor_scalar(out=actb[:, mc * 256:(mc + 1) * 256],
                                    in0=act_ps[:], scalar1=1.0 / 1024.0,
                                    scalar2=0.0, op0=STT.mult, op1=STT.max)
            if debug and d == 0:
                t = dbg_out('dbg_actb', [128, 4096], dt.float8e4)
                nc.sync.dma_start(out=t[:], in_=actb[:])

            # --- H: Wo accumulate into xsad ---
            actb_r = actb[:].rearrange('p (k j) -> p k j', k=16)
            for kcp in range(8):
                wor = wo[kcp][:].rearrange('p (i x) -> p i x', i=2)
                for mc in range(2):
                    mm(xsad_ps[:, mc * 256:(mc + 1) * 256],
                       wor[:, :, mc * 128:(mc + 1) * 128],
                       actb_r[:, 2 * kcp:2 * kcp + 2, :],
                       start=False, stop=(kcp == 7), perf_mode=DR)

            # --- I: norm, partition-major stats ---
            # u true = xsad_ps / 2^17
            u_bf = sb.tile([128, 512], dt.bfloat16, tag='u_bf', bufs=1,
                           name='u_bf')
            act(out=u_bf[:], in_=xsad_ps[:], func=AF.Copy,
                scale=1.0 / 131072.0)
            u2 = sb.tile([128, 512], dt.bfloat16, tag='u2', bufs=1, name='u2')
            V.tensor_mul(u2[:], u_bf[:], u_bf[:])
            xu = sb.tile([128, 512], dt.bfloat16, tag='xu', bufs=1, name='xu')
            V.tensor_mul(xu[:], master[:], u_bf[:])
            # stats psum (128, 6): [su0 su1 qu0 qu1 c0 c1]
            nrm_ps = pps.tile([128, 512], dt.float32, tag='nrm', bufs=1,
                              name='nrm_ps')
            stp = nrm_ps[:, 0:6]
            for si, s in enumerate((u_bf, u2, xu)):
                for jb in range(2):
                    for ec in range(2):
                        mm(stp[:, si * 2 + jb: si * 2 + jb + 1],
                           s[:, ec * 256 + jb * 128: ec * 256 + jb * 128 + 128],
                           ones_cb[:], start=(ec == 0), stop=(ec == 1))


            def row(nm):
                return rows.tile([128, 2], dt.float32, tag='rw', bufs=24,
                                 name=nm)

            stq = rows.tile([128, 6], dt.float32, tag='stq', bufs=2,
                            name='stq')
            V.tensor_copy(out=stq[:], in_=stp)
            su, qu, cc = stq[:, 0:2], stq[:, 2:4], stq[:, 4:6]
            bc_ps = nrm_ps
            t3, t5 = row('t3'), row('t5')
            V.scalar_tensor_tensor(out=t3[:], in0=su, scalar=-1.0 / E,
                                   in1=su, op0=STT.mult, op1=STT.mult)
            V.tensor_add(t5[:], t3[:], qu)
            stdu = row('stdu')
            act(out=stdu[:], in_=t5[:], func=AF.Sqrt, scale=1.0 / (E - 1))
            s1p, s1, alpha = row('s1p'), row('s1'), row('alpha')
            V.tensor_scalar_add(out=s1p[:], in0=stdu[:], scalar1=1.0)
            V.reciprocal(s1[:], s1p[:])
            V.tensor_scalar_mul(out=alpha[:], in0=s1[:], scalar1=STEP)
            asu, sy = row('asu'), row('sy')
            V.tensor_mul(asu[:], alpha[:], su)
            V.tensor_add(sy[:], asu[:], st_carry[:, 0:2])
            ac2, aa, aqu, qy0, qy = (row('ac2'), row('aa'), row('aqu'),
                                     row('qy0'), row('qy'))
            V.scalar_tensor_tensor(out=ac2[:], in0=alpha[:], scalar=2.0,
                                   in1=cc, op0=STT.mult, op1=STT.mult)
            V.tensor_mul(aa[:], alpha[:], alpha[:])
            V.tensor_mul(aqu[:], aa[:], qu)
            V.tensor_add(qy0[:], ac2[:], st_carry[:, 2:4])
            V.tensor_add(qy[:], qy0[:], aqu[:])
            t4, t5b = row('t4'), row('t5b')
            V.scalar_tensor_tensor(out=t4[:], in0=sy[:], scalar=-1.0 / E,
                                   in1=sy[:], op0=STT.mult, op1=STT.mult)
            V.tensor_add(t5b[:], t4[:], qy[:])
            stdy = row('stdy')
            act(out=stdy[:], in_=t5b[:], func=AF.Sqrt, scale=1.0 / (E - 1))
            s2p, s2, as2 = row('s2p'), row('s2'), row('as2')
            V.tensor_scalar_add(out=s2p[:], in0=stdy[:], scalar1=1.0)
            V.reciprocal(s2[:], s2p[:])
            V.tensor_mul(as2[:], alpha[:], s2[:])
            # carried stats for next layer (off critical path)
            st_carry = mpool.tile([128, 4], dt.float32, tag='stc', name='stc')
            s2sq = row('s2sq')
            V.tensor_mul(st_carry[:, 0:2], sy[:], s2[:])
            V.tensor_mul(s2sq[:], s2[:], s2[:])
            V.tensor_mul(st_carry[:, 2:4], qy[:], s2sq[:])
            # broadcast s2 / as2 over partitions via diag matmul (reuses the
            # nrm bank; the tile framework serializes it after the stq read)
            for vi, vv in enumerate((s2, as2)):
                for jb in range(2):
                    dgn = rows.tile([128, 128], dt.bfloat16, tag='dgn',
                                    bufs=4, name='dgn')
                    V.tensor_scalar_mul(out=dgn[:], in0=ident[:],
                                        scalar1=vv[:, jb:jb + 1])
                    mm(bc_ps[:, vi * 256 + jb * 128: vi * 256 + jb * 128 + 128],
                       ones128[:], dgn[:], start=True, stop=True)
            newmaster = mpool.tile([128, 512], dt.float32, tag='master',
                                   name='master')
            for ec in range(2):
                ta = sb.tile([128, 256], dt.float32, tag='tmp', bufs=2,
                             name='ta')
                V.tensor_mul(ta[:], u_bf[:, ec * 256:(ec + 1) * 256],
                             bc_ps[:, 256:512])
                V.tensor_mul(newmaster[:, ec * 256:(ec + 1) * 256],
                             master[:, ec * 256:(ec + 1) * 256],
                             bc_ps[:, 0:256])
                V.tensor_add(newmaster[:, ec * 256:(ec + 1) * 256],
                             newmaster[:, ec * 256:(ec + 1) * 256], ta[:])
            master = newmaster
            loc_bf = mpool.tile([128, 512], dt.bfloat16, tag='locb',
                                name='locb')
            loc = mpool.tile([128, 512], dt.float8e4, tag='loc', name='loc')
            for ec in range(2):
                sl = slice(ec * 256, (ec + 1) * 256)
                act(out=loc_bf[:, sl], in_=master[:, sl], func=AF.Copy)
                V.tensor_scalar_mul(out=loc[:, sl], in0=master[:, sl],
                                    scalar1=SA)
            if debug:
                t = dbg_out(f'dbg_xsa{d}', [128, 512], dt.float32)
                nc.sync.dma_start(out=t[:], in_=master[:])

        # ================= HEAD =================
        full_bf, full = comm_gather(loc_bf)
        full_r = full[:].rearrange('p (e g) -> p e g', e=2)
        nat = nat_transpose(full_bf)

        # lptok: (e, j) per pair batch; fp8 x256
        lptok = hp.tile([128, 128], dt.float8e4, name='lptok')
        for ec in range(2):
            l_ps = ppy.tile([128, 64], dt.float32, tag='aps', name='l_ps')
            for lb in range(4):
                mm(l_ps[:], nat[:, lb * 256 + ec * 128: lb * 256 + ec * 128 + 128],
                   msel[:, lb * 64:(lb + 1) * 64],
                   start=(lb == 0), stop=(lb == 3))
            V.tensor_copy(out=lptok[:, ec * 64:(ec + 1) * 64], in_=l_ps[:])

        # xx: kchoice (e, n) n = j*4+kn; psum = 2^17 true; out fp8 x256
        xxsb = hp.tile([128, 512], dt.float8e4, name='xxsb')
        for kn in range(KN):
            for ec in range(2):
                x_ps = ppy.tile([128, 64], dt.float32, tag='aps', name='x_ps')
                for fc in range(2):
                    off = (fc * 8 + kn * 2 + ec) * 128
                    mm(x_ps[:], wkct[:, off:off + 128],
                       lptok[:, fc * 64:(fc + 1) * 64],
                       start=(fc == 0), stop=(fc == 1))
                dst = xxsb[:, ec * 256:(ec + 1) * 256].rearrange(
                    'p (j f) -> p f j', f=4)[:, kn, :]
                nc.vector.tensor_scalar(
                    out=dst, in0=x_ps[:],
                    scalar1=bkc_sb[:, kn * 2 + ec: kn * 2 + ec + 1],
                    scalar2=1.0 / 512.0, op0=STT.add, op1=STT.mult)

        # xx2T: (l, n) blocks; fp8 x256
        xxsb_r = xxsb[:].rearrange('p (e n) -> p e n', e=2)
        xx2 = hp.tile([128, 1024], dt.float8e4, name='xx2')
        for lb in range(4):
            x_ps = ppy.tile([128, 256], dt.float32, tag='aps', name='x2_ps')
            mm(x_ps[:], full_r[:, :, lb * 128:(lb + 1) * 128], xxsb_r,
               start=True, stop=True, perf_mode=DR)
            V.tensor_scalar_mul(out=xx2[:, lb * 256:(lb + 1) * 256],
                                in0=x_ps[:], scalar1=1.0 / 256.0)

        # xx3T: (e, n); fp8 x256
        xx2_r = xx2[:].rearrange('p (kb n) -> p kb n', kb=4)
        nath_r = nat[:].rearrange('p (kb x) -> p kb x', kb=4)
        xx3 = hp.tile([128, 512], dt.float8e4, name='xx3')
        for ec in range(2):
            x_ps = ppy.tile([128, 256], dt.float32, tag='aps', name='x3_ps')
            for kbp in range(2):
                mm(x_ps[:],
                   nath_r[:, 2 * kbp:2 * kbp + 2, ec * 128:(ec + 1) * 128],
                   xx2_r[:, 2 * kbp:2 * kbp + 2, :],
                   start=(kbp == 0), stop=(kbp == 1), perf_mode=DR)
            V.tensor_scalar_mul(out=xx3[:, ec * 256:(ec + 1) * 256],
                                in0=x_ps[:], scalar1=1.0 / 256.0)

        # xxWT: (e, n); fp8 x256
        wem_r = wem[:].rearrange('p (k x) -> p k x', k=2)
        xx3_r = xx3[:].rearrange('p (k n) -> p k n', k=2)
        xxw = hp.tile([128, 512], dt.float8e4, name='xxw')
        for ec in range(2):
            x_ps = ppy.tile([128, 256], dt.float32, tag='aps', name='xw_ps')
            mm(x_ps[:], wem_r[:, :, ec * 128:(ec + 1) * 128], xx3_r,
               start=True, stop=True, perf_mode=DR)
            V.tensor_scalar_mul(out=xxw[:, ec * 256:(ec + 1) * 256],
                                in0=x_ps[:], scalar1=1.0 / 512.0)

        # all-gather xxW across batches (fp8)
        xxw_in = dram.tile([2, 128, 256], dt.float8e4, tag='xxw_in')
        xxw_out = dram.tile([8, 128, 256], dt.float8e4, tag='xxw_out')
        for ec in range(2):
            nc.sync.dma_start(out=xxw_in[ec], in_=xxw[:, ec * 256:(ec + 1) * 256])
        nc.gpsimd.collective_compute(
            "AllGather", mybir.AluOpType.bypass,
            replica_groups=[[0, 2, 4, 6], [1, 3, 5, 7]],
            ins=[xxw_in.opt()], outs=[xxw_out.opt()],
        )
        # xxwall: (128, 2, 1024) fp8: [p, ec, n] n = b*256 + j*4 + kn
        xxwall = hp.tile([128, 2048], dt.float8e4, name='xxwall')
        xxwall_r = xxwall[:].rearrange('p (e n) -> p e n', e=2)
        for bb in range(4):
            for ec in range(2):
                nc.sync.dma_start(
                    out=xxwall_r[:, ec, bb * 256:(bb + 1) * 256],
                    in_=xxw_out[bb * 2 + ec])
        if debug:
            t = dbg_out('dbg_lptok', [128, 128], dt.float8e4)
            nc.sync.dma_start(out=t[:], in_=lptok[:])
            t = dbg_out('dbg_xx', [128, 512], dt.float8e4)
            nc.sync.dma_start(out=t[:], in_=xxsb[:])
            t = dbg_out('dbg_xx2', [128, 1024], dt.float8e4)
            nc.sync.dma_start(out=t[:], in_=xx2[:])
            t = dbg_out('dbg_xx3', [128, 512], dt.float8e4)
            nc.sync.dma_start(out=t[:], in_=xx3[:])
            t = dbg_out('dbg_xxwall', [128, 2048], dt.float8e4)
            nc.sync.dma_start(out=t[:], in_=xxwall[:])

        # clog: per-row dot of xxW with target embedding (tb = 2^17 true)
        tb = hp.tile([128, 2048], dt.bfloat16, name='tb')
        ett_r = ett[:].rearrange('p (e n) -> p e n', e=2)
        for ec in range(2):
            V.tensor_mul(tb[:].rearrange('p (e n) -> p e n', e=2)[:, ec, :],
                         xxwall_r[:, ec, :], ett_r[:, ec, :])
        clog_d = dram.tile([1024], dt.float32, tag='clog_d')
        tb_r = tb[:].rearrange('p (e n) -> p e n', e=2)
        for half in range(2):
            cl_t = pps.tile([128, 512], dt.float32, tag='nrm', bufs=1,
                            name='cl_t')
            cl_ps = cl_t[0:1, :]
            for ec in range(2):
                mm(cl_ps, ones_cb[:],
                   tb_r[:, ec, half * 512:(half + 1) * 512],
                   start=(ec == 0), stop=(ec == 1))
            cl_sb = hp.tile([1, 512], dt.float32, tag='cl_sb', bufs=2,
                            name='cl_sb')
            act(out=cl_sb[:], in_=cl_ps[:], func=AF.Copy,
                scale=1.0 / 131072.0)
            nc.sync.dma_start(out=clog_d[half * 512:(half + 1) * 512],
                              in_=cl_sb[:])
        if debug:
            t = dbg_out('dbg_clog', [1024], dt.float32)
            nc.sync.dma_start(out=t[:], in_=clog_d[:])

        # logits + per-shard sum-exp (vocab parallel, fp8 DoubleRow),
        # two row-half passes; each half's stats AllGather overlaps the rest
        st_outs = []
        for half in range(2):
            stats = hp.tile([128, 32], dt.float32, tag='hstats', bufs=2,
                            name='stats')
            for vc in range(NVC):
                er = embt[vc][:].rearrange('p (e n) -> p e n', e=2)
                for nbh in range(4):
                    nb = half * 4 + nbh
                    if nbh % 2 == 0:
                        lg_ps = ppq.tile([128, VC], dt.float32, tag='qps',
                                         name='lg_ps')
                    else:
                        lg_ps = pst.tile([128, VC], dt.float32, tag='sT',
                                         bufs=2, name='lg_ps2')
                    mm(lg_ps[:], xxwall_r[:, :, nb * 128:(nb + 1) * 128], er,
                       start=True, stop=True, perf_mode=DR)
                    escr = hp.tile([128, VC], dt.bfloat16, tag='escr', bufs=2,
                                   name='escr')
                    act(out=escr[:], in_=lg_ps[:], func=AF.Exp,
                        scale=1.0 / 131072.0)
                    V.reduce_sum(out=stats[:, nbh * 8 + vc: nbh * 8 + vc + 1],
                                 in_=escr[:], axis=AX.X)
            se = hp.tile([128, 4], dt.float32, tag='se', bufs=2, name='se')
            for nbh in range(4):
                V.reduce_sum(out=se[:, nbh:nbh + 1],
                             in_=stats[:, nbh * 8:(nbh + 1) * 8], axis=AX.X)
            st_in = dram.tile([512], dt.float32, tag='st_in')
            st_out = dram.tile([512], dt.float32, tag='st_out',
                               addr_space="Shared")
            nc.gpsimd.dma_start(
                out=st_in[:].rearrange('(nb p) -> p nb', p=128), in_=se[:])
            nc.gpsimd.collective_compute(
                "AllReduce", mybir.AluOpType.add,
                replica_groups=[list(range(NCORES))],
                ins=[st_in.opt()], outs=[st_out.opt()],
            )
            st_outs.append(st_out)

        # combine gathered partial sum-exps + cent + weighted sum
        cent = hp.tile([128, 2], dt.float32, name='cent')
        lse_g = hp.tile([128, 8], dt.float32, tag='lse_g', name='lse_g')
        cg = hp.tile([128, 8], dt.float32, tag='cg', name='cg')
        for t_ in range(2):
            nc.sync.dma_start(
                out=lse_g[:, t_ * 4:(t_ + 1) * 4],
                in_=st_outs[t_][:].rearrange('(p f) -> p f', f=4))
            nc.sync.dma_start(
                out=cg[:, t_ * 4:(t_ + 1) * 4],
                in_=clog_d[t_ * 512:(t_ + 1) * 512].rearrange(
                    '(p f) -> p f', f=4))
        lse = hp.tile([128, 8], dt.float32, tag='lse', name='lse')
        act(out=lse[:], in_=lse_g[:], func=AF.Ln)
        df = hp.tile([128, 8], dt.float32, tag='df', name='df')
        V.tensor_sub(df[:], cg[:], lse[:])
        ex = hp.tile([128, 8], dt.float32, tag='ex', name='ex')
        act(out=ex[:], in_=df[:], func=AF.Exp)
        for t_ in range(2):
            sm = hp.tile([128, 1], dt.float32, tag='sm', bufs=2, name='sm')
            V.reduce_sum(out=sm[:], in_=ex[:, t_ * 4:(t_ + 1) * 4], axis=AX.X)
            act(out=cent[:, t_:t_ + 1], in_=sm[:], func=AF.Ln)
        num_t = pps.tile([128, 512], dt.float32, tag='nrm', bufs=1,
                         name='num_t')
        num_ps = num_t[0:4, 0:1]
        for t_ in range(2):
            mm(num_ps, wind[:, t_ * 4:(t_ + 1) * 4], cent[:, t_:t_ + 1],
               start=(t_ == 0), stop=(t_ == 1))
        outsb = hp.tile([4, 1], dt.float32, name='outsb')
        V.tensor_copy(out=outsb[:], in_=num_ps)
        nc.sync.dma_start(out=out_t[:], in_=outsb[:])
        if debug:
            t = dbg_out('dbg_cent', [128, 2], dt.float32)
            nc.sync.dma_start(out=t[:], in_=cent[:])

    nc.compile()
    return nc


def kernel(**inputs):
    from concourse.bass_utils import run_bass_kernel_spmd

    in_maps, aux = _prep(inputs)
    key = (_D_EFF, _DEBUG)
    if key not in _CACHE:
        _CACHE[key] = _build(_D_EFF, _DEBUG)
    nc = _CACHE[key]
    res = run_bass_kernel_spmd(nc, in_maps, list(range(NCORES)), trace=_TRACE)
    kernel._last_results = res
    num = np.asarray(res.results[0]['out'], np.float32)[:, 0]
    summer = aux['summer']
    sumw = summer.sum(-1)
    loss = -(num - np.log(KN) * sumw) / np.clip(sumw, 1.0, None)
    return loss.astype(np.float32)


# revision 23
# speedup vs baseline: 1.3026x; 1.3026x over previous
"""Distributed Trainium2 Bass kernel for nn_AddModelWithAttentionStacked.

Sharding: mesh B(4) x L(2) over 8 NeuronCores. Core c owns batch b=c//2 and
sequence rows [r0, r0+256) with r0 = (c%2)*256. Activations are kept
feature-major (E on partitions) in SBUF. Per layer the pair all-gathers the
updated xsa shard (keys/values for attention); the loss head is
vocab-parallel over all 8 cores (per-shard sum-exp + AllGather of partials).

Perf structure vs the original baseline:
- fp8(e4m3) DoubleRow matmuls everywhere heavy (2x PE throughput), with
  static power-of-two scales chosen from measured magnitudes.
- transposed attention scores (queries on partitions): softmax sums on the
  free axis, normalization folded into the PE transpose via diag(1/sum).
- sequence rolls as shifted DVE copies (+ masked boundary blend) instead of
  shift matmuls.
- layer norm stats kept partition-major (row index on partitions) so the
  whole stats chain is ~60ns DVE ops instead of 2us 1-partition ops.
- loss-head sum-exp combined via AllGather + local reduce (cheaper floor
  than AllReduce), fp8 logits, all embed tiles persistent in SBUF.
"""

import numpy as np
import ml_dtypes

G, E, K, D, B, L, M, KN = 32000, 256, 8, 6, 4, 512, 64, 4
STEP, EPS = 0.05, 1.0
NCORES = 8
RL = L // 2          # 256 local rows
VS = G // NCORES     # 4000 vocab shard
VC = 500             # vocab chunk
NVC = VS // VC       # 8

_D_EFF = D
_DEBUG = False
_TRACE = False
_CACHE = {}

f8np = ml_dtypes.float8_e4m3
bf16np = ml_dtypes.bfloat16

# static scales (power of two), from measured magnitudes:
#   xsa max 0.31, q max 0.39, a1/a2 max 0.32, yik max 0.013, actb max 0.027,
#   weights max 0.28, xx/xx2/xx3/xxw max 0.53, probs ~= 1/512.
SA = 256.0     # activations: loc/full/nat/rp1/rm1/a1/a2/qT/lptok/xx*/embt-in
SW = 512.0     # weights: WTC/WQT/WDT/WKCT/WEM/ETT/EMBT
SWO = 32.0     # Wo
SXID = 8192.0  # yik
SACT = 4096.0  # relu(Wd @ xid)
SEST = 32768.0  # softmax probs


def _f8(x, s):
    return np.ascontiguousarray(
        np.clip(np.asarray(x, np.float32) * s, -240.0, 240.0).astype(f8np))


def _bf(x):
    return np.ascontiguousarray(np.asarray(x, np.float32).astype(bf16np))


def _f32(x):
    return np.ascontiguousarray(np.asarray(x, np.float32))


def _norm_np(x):
    return x / (EPS + np.std(x, axis=-1, ddof=1, keepdims=True))


def _prep(inputs):
    masked = np.asarray(inputs['masked'])
    unmasked = np.asarray(inputs['unmasked'])
    mask = np.asarray(inputs['mask'])
    summer = np.asarray(inputs['summer'], np.float32)
    embed = np.asarray(inputs['embed'], np.float32)
    pos = np.asarray(inputs['pos'], np.float32)
    Wt = np.asarray(inputs['Wt'], np.float32)
    Wc = np.asarray(inputs['Wc'], np.float32)
    Wq = np.asarray(inputs['Wq'], np.float32)
    Wd = np.asarray(inputs['Wd'], np.float32)
    Wo = np.asarray(inputs['Wo'], np.float32)
    Wkc = np.asarray(inputs['Wkc'], np.float32)
    bkc = np.asarray(inputs['bkc'], np.float32)
    Wem = np.asarray(inputs['Wem'], np.float32)

    # ---- shared (identical on all cores) ----
    # WTC: [d, p, mat(4), kc(2), mc(2), c(128)] -> flat (D, 128, 2048)
    def blk_nat(w):  # [d, p, kc, mc, c] = w[d, kc*128+p, mc*128+c]
        return w.reshape(D, 2, 128, 2, 128).transpose(0, 2, 1, 3, 4)

    def blk_tr(w):   # [d, p, kc, mc, c] = w[d, mc*128+c, kc*128+p]
        return w.reshape(D, 2, 128, 2, 128).transpose(0, 4, 3, 1, 2)

    wtc = np.stack([blk_nat(Wt), blk_nat(Wc), blk_tr(Wc), blk_tr(Wt)], axis=2)
    WTC = _f8(wtc.reshape(D, 128, 4 * 2 * 2 * 128), SW)

    # WQT: [d, p, kc(2), mc(16), c] = Wq[d, mc*128+c, kc*128+p]
    wq = Wq.reshape(D, 16, 128, 2, 128).transpose(0, 4, 3, 1, 2)
    WQT = _f8(wq.reshape(D, 128, 2 * 16 * 128), SW)

    # WDT: [d, kcp(8), p, i(2), mc(16), c] = Wd[d, mc*128+c, (2kcp+i)*128+p]
    wd = Wd.reshape(D, 16, 128, 16, 128).transpose(0, 3, 4, 1, 2)
    wd = wd.reshape(D, 8, 2, 128, 16, 128).transpose(0, 1, 3, 2, 4, 5)
    WDT = _f8(wd.reshape(D, 8, 128, 2 * 16 * 128), SW)

    # WO: [d, kcp(8), p, i(2), mc(2), c] = Wo[d, (2kcp+i)*128+p, mc*128+c]
    wo = Wo.reshape(D, 8, 2, 128, 2, 128).transpose(0, 1, 3, 2, 4, 5)
    WO = _f8(wo.reshape(D, 8, 128, 2 * 2 * 128), SWO)

    # WKCT: [p, fc(2), knec(8), c] = Wkc[knec*128+c, fc*128+p]
    wk = Wkc.reshape(8, 128, 2, 128).transpose(3, 2, 0, 1)
    WKCT = _f8(wk.reshape(128, 2 * 8 * 128), SW)

    # WEM: [p, kc(2), ec(2), c] = Wem[kc*128+p, ec*128+c]
    we = Wem.reshape(2, 128, 2, 128).transpose(1, 0, 2, 3)
    WEM = _f8(we.reshape(128, 2 * 2 * 128), SW)

    BKC = _f32(bkc.reshape(8, 128).T * (SW * SA))  # (128, 8) [p, knec]

    # ---- derived host math ----
    xsa0 = _norm_np(embed[masked] + pos[None])  # (B, L, E) f32
    tgt = np.take_along_axis(unmasked, mask, axis=1)  # (B, M)
    # ETT: [p, ec(2), n(1024)] = embed[tgt[b, j], ec*128+p], n = b*256+j*4+kn
    tgt_rep = np.repeat(tgt.reshape(B * M), KN)
    ett = embed[tgt_rep]  # (1024, 256)
    ETT = _f8(ett.reshape(1024, 2, 128).transpose(2, 1, 0).reshape(128, 2048),
              SW)

    # WIND: [i, t(2), bcol(4)]: r = t*128+i = b*64+m -> summer[b, m]*(bcol==b)
    wind = np.zeros((128, 2, B), np.float32)
    r = np.arange(256)
    bi, mi = r // M, r % M
    wind[r % 128, r // 128, bi] = summer[bi, mi]
    WIND = _f32(wind.reshape(128, 2 * B))

    shared = dict(WTC=WTC, WQT=WQT, WDT=WDT, WO=WO, WKCT=WKCT, WEM=WEM,
                  BKC=BKC, ETT=ETT, WIND=WIND)

    # ---- per-core ----
    in_maps = []
    for c in range(NCORES):
        b, h = c // 2, c % 2
        r0 = h * RL
        m = dict(shared)
        # XSA0: (128, 2*256) f32: [p, ec*256+j] = xsa0[b, r0+j, ec*128+p]
        x0 = xsa0[b, r0:r0 + RL]  # (256, 256)
        m['XSA0'] = _f32(x0.reshape(RL, 2, 128).transpose(2, 1, 0)
                         .reshape(128, 512))
        # FULL0: whole pair batch, feature-major: [p, ec*512+g]
        m['FULL0'] = _bf(xsa0[b].reshape(L, 2, 128).transpose(2, 1, 0)
                         .reshape(128, 1024))
        # SC0 partition-major: (128, 4) [jp, (sx0, sx1, qx0, qx1)]
        su = x0.sum(-1).reshape(2, 128)      # [jb, jp]
        qu = (x0 * x0).sum(-1).reshape(2, 128)
        m['SC0'] = _f32(np.stack([su[0], su[1], qu[0], qu[1]], axis=1))
        # MSEL: [p, lb(4), j(64)] = 1 iff lb*128+p == mask[b, j]   (fp8)
        ms = np.zeros((L, M), np.float32)
        ms[mask[b], np.arange(M)] = 1.0
        m['MSEL'] = _f8(ms.reshape(4, 128, M).transpose(1, 0, 2)
                        .reshape(128, 4 * M), 1.0)
        # PM: (128, 2) f32: col0 = 1 if partner is rank 0 of pair (h==1)
        pm = np.zeros((128, 2), np.float32)
        pm[:, 0] = SA if h == 1 else 0.0
        pm[:, 1] = SA if h == 0 else 0.0
        m['PM'] = _f32(pm)
        # EMBT: [vc(8), p, ec(2), n(500)] = embed[c*4000+vc*500+n, ec*128+p]
        shard = embed[c * VS:(c + 1) * VS]  # (4000, 256)
        et = shard.reshape(NVC, VC, 2, 128).transpose(0, 3, 2, 1)
        m['EMBT'] = _f8(et.reshape(NVC, 128, 2 * VC), SW)
        in_maps.append(m)

    aux = dict(summer=summer)
    return in_maps, aux


def _build(d_eff, debug):
    import concourse.bass as bass
    import concourse.tile as tile
    from concourse import mybir, bacc
    from concourse.masks import make_identity
    from contextlib import ExitStack

    dt = mybir.dt
    AF = mybir.ActivationFunctionType
    AX = mybir.AxisListType
    STT = mybir.AluOpType
    DR = mybir.MatmulPerfMode.DoubleRow

    nc = bacc.Bacc("TRN2", num_devices=NCORES)

    def par(name, shape, dtype=dt.float8e4):
        return nc.dram_tensor(name, shape, dtype, kind="ExternalInput")

    P = {}
    P['WTC'] = par('WTC', [D, 128, 2048])
    P['WQT'] = par('WQT', [D, 128, 4096])
    P['WDT'] = par('WDT', [D, 8, 128, 4096])
    P['WO'] = par('WO', [D, 8, 128, 512])
    P['WKCT'] = par('WKCT', [128, 2048])
    P['WEM'] = par('WEM', [128, 512])
    P['BKC'] = par('BKC', [128, 8], dt.float32)
    P['ETT'] = par('ETT', [128, 2048])
    P['WIND'] = par('WIND', [128, 8], dt.float32)
    P['XSA0'] = par('XSA0', [128, 512], dt.float32)
    P['FULL0'] = par('FULL0', [128, 1024], dt.bfloat16)
    P['SC0'] = par('SC0', [128, 4], dt.float32)
    P['MSEL'] = par('MSEL', [128, 256])
    P['PM'] = par('PM', [128, 2], dt.float32)
    P['EMBT'] = par('EMBT', [NVC, 128, 1000])

    out_t = nc.dram_tensor("out", [4, 1], dt.float32, kind="ExternalOutput")
    dbg = {}

    def dbg_out(name, shape, dtype):
        if debug and name not in dbg:
            dbg[name] = nc.dram_tensor(name, shape, dtype,
                                       kind="ExternalOutput")
        return dbg.get(name)

    with tile.TileContext(nc) as tc, ExitStack() as ctx:
        con = ctx.enter_context(tc.tile_pool(name="con", bufs=1))
        pers = ctx.enter_context(tc.tile_pool(name="pers", bufs=1))
        sb = ctx.enter_context(tc.tile_pool(name="sb", bufs=2))
        mpool = ctx.enter_context(tc.tile_pool(name="mpool", bufs=2))
        wdp = ctx.enter_context(tc.tile_pool(name="wdp", bufs=24))
        rows = ctx.enter_context(tc.tile_pool(name="rows", bufs=1))
        hp = ctx.enter_context(tc.tile_pool(name="hp", bufs=1))
        ppx = ctx.enter_context(tc.tile_pool(name="ppx", bufs=1, space="PSUM"))
        pst = ctx.enter_context(tc.tile_pool(name="pst", bufs=2, space="PSUM"))
        ptp = ctx.enter_context(tc.tile_pool(name="ptp", bufs=1, space="PSUM"))
        ppq = ctx.enter_context(tc.tile_pool(name="ppq", bufs=1, space="PSUM"))
        ppy = ctx.enter_context(tc.tile_pool(name="ppy", bufs=2, space="PSUM"))
        pps = ctx.enter_context(tc.tile_pool(name="pps", bufs=1, space="PSUM"))
        dram = ctx.enter_context(tc.tile_pool(name="dram", bufs=2,
                                              space="DRAM"))

        mm = nc.tensor.matmul
        act = nc.scalar.activation
        V = nc.vector

        # rendezvous: tiny all-reduce so core-start skew is absorbed here
        rdv_in = dram.tile([128], dt.float32, tag='rdv_in')
        rdv_out = dram.tile([128], dt.float32, tag='rdv_out')
        rdv_sb = con.tile([1, 128], dt.float32)
        V.memset(rdv_sb, 0.0)
        nc.gpsimd.dma_start(out=rdv_in[:], in_=rdv_sb[:])
        nc.gpsimd.collective_compute(
            "AllReduce", mybir.AluOpType.add,
            replica_groups=[list(range(NCORES))],
            ins=[rdv_in.opt()], outs=[rdv_out.opt()],
        )

        # master xsa (feature-major f32); layer-0 gather comes from the host
        master = mpool.tile([128, 512], dt.float32, tag='master')
        nc.sync.dma_start(out=master[:], in_=P['XSA0'][:])
        loc_bf = mpool.tile([128, 512], dt.bfloat16, tag='locb')
        act(out=loc_bf[:], in_=master[:], func=AF.Copy)
        loc = mpool.tile([128, 512], dt.float8e4, tag='loc')
        V.tensor_scalar_mul(out=loc[:], in0=master[:], scalar1=SA)

        def comm_gather(loc_t):
            """AllGather pair's xsa (bf16) -> full_bf (128, 2*512) + fp8 copy."""
            ag_in = dram.tile([2, 128, 256], dt.bfloat16, tag='ag_in')
            ag_out = dram.tile([4, 128, 256], dt.bfloat16, tag='ag_out')
            for ec in range(2):
                nc.gpsimd.dma_start(out=ag_in[ec],
                                    in_=loc_t[:, ec * 256:(ec + 1) * 256])
            nc.gpsimd.collective_compute(
                "AllGather", mybir.AluOpType.bypass,
                replica_groups=[[0, 1], [2, 3], [4, 5], [6, 7]],
                ins=[ag_in.opt()], outs=[ag_out.opt()],
            )
            full_bf = sb.tile([128, 1024], dt.bfloat16, tag='fullb',
                              name='full_bf')
            for r in range(2):
                for ec in range(2):
                    nc.gpsimd.dma_start(
                        out=full_bf[:, ec * 512 + r * 256: ec * 512 + r * 256 + 256],
                        in_=ag_out[r * 2 + ec])
            full = sb.tile([128, 1024], dt.float8e4, tag='full', name='full')
            V.tensor_scalar_mul(out=full[:, 0:512], in0=full_bf[:, 0:512],
                                scalar1=SA)
            V.tensor_scalar_mul(out=full[:, 512:1024],
                                in0=full_bf[:, 512:1024], scalar1=SA)
            return full_bf, full

        full_bf0 = sb.tile([128, 1024], dt.bfloat16, tag='fullb',
                           name='full_bf')
        nc.sync.dma_start(out=full_bf0[:], in_=P['FULL0'][:])
        full8_0 = sb.tile([128, 1024], dt.float8e4, tag='full', name='full')
        V.tensor_scalar_mul(out=full8_0[:, 0:512], in0=full_bf0[:, 0:512],
                            scalar1=SA)
        V.tensor_scalar_mul(out=full8_0[:, 512:1024],
                            in0=full_bf0[:, 512:1024], scalar1=SA)
        full0 = (full_bf0, full8_0)

        # constants
        ident = con.tile([128, 128], dt.bfloat16)
        make_identity(nc, ident)
        ident8 = con.tile([128, 128], dt.float8e4)
        V.tensor_copy(out=ident8[:], in_=ident[:])
        ones128 = con.tile([128, 128], dt.bfloat16)
        V.memset(ones128, 1.0)
        ones_cb = con.tile([128, 1], dt.bfloat16)
        V.memset(ones_cb, 1.0)

        # persistent inputs (loaded once; EMBT persistent for both passes)
        msel = pers.tile([128, 256], dt.float8e4)
        nc.sync.dma_start(out=msel[:], in_=P['MSEL'][:])
        wkct = pers.tile([128, 2048], dt.float8e4)
        nc.sync.dma_start(out=wkct[:], in_=P['WKCT'][:])
        wem = pers.tile([128, 512], dt.float8e4)
        nc.sync.dma_start(out=wem[:], in_=P['WEM'][:])
        bkc_sb = pers.tile([128, 8], dt.float32)
        nc.sync.dma_start(out=bkc_sb[:], in_=P['BKC'][:])
        ett = pers.tile([128, 2048], dt.float8e4)
        nc.sync.dma_start(out=ett[:], in_=P['ETT'][:])
        wind = pers.tile([128, 8], dt.float32)
        nc.sync.dma_start(out=wind[:], in_=P['WIND'][:])
        pmask = pers.tile([128, 2], dt.float32)
        nc.sync.dma_start(out=pmask[:], in_=P['PM'][:])
        embt = [pers.tile([128, 1000], dt.float8e4, name=f'embt{vc}')
                for vc in range(NVC)]
        for vc in range(NVC):
            nc.sync.dma_start(out=embt[vc][:], in_=P['EMBT'][vc])
        st_carry = mpool.tile([128, 4], dt.float32, tag='stc')
        nc.sync.dma_start(out=st_carry[:], in_=P['SC0'][:])

        def nat_transpose(full_bf):
            """nat (128, 1024) fp8 xSA: [keys_p, kb*256 + ec*128 + c]."""
            nat = sb.tile([128, 1024], dt.float8e4, tag='nat', name='nat')
            nat_4d = nat[:].rearrange('p (kb e c) -> p kb e c', kb=4, e=2)
            for ec in range(2):
                tp = ptp.tile([128, 512], dt.bfloat16, tag='tp', bufs=1,
                              name='tp')
                for kb in range(4):
                    nc.tensor.transpose(
                        tp[:, kb * 128:(kb + 1) * 128],
                        full_bf[:, ec * 512 + kb * 128: ec * 512 + kb * 128 + 128],
                        ident[:])
                if ec == 0:
                    V.tensor_scalar_mul(
                        out=nat_4d[:, :, ec, :],
                        in0=tp[:].rearrange('p (kb c) -> p kb c', kb=4),
                        scalar1=SA)
                else:
                    act(out=nat_4d[:, :, ec, :],
                        in_=tp[:].rearrange('p (kb c) -> p kb c', kb=4),
                        func=AF.Copy, scale=SA)
            return nat

        for d in range(d_eff):
            # --- A: pair all-gather of xsa (issued first; qT overlaps it) ---
            full_bf, full = full0 if d == 0 else comm_gather(loc_bf)
            full_r = full[:].rearrange('p (e g) -> p e g', e=2)

            # --- B: qT projection (local only; overlaps the collective) ---
            wq = sb.tile([128, 4096], dt.float8e4, tag='wq', bufs=2, name='wq')
            nc.sync.dma_start(out=wq[:], in_=P['WQT'][d])
            wq_r = wq[:].rearrange('p (k x) -> p k x', k=2)
            loc_r = loc[:].rearrange('p (e j) -> p e j', e=2)
            qT = sb.tile([128, 4096], dt.float8e4, tag='qT', bufs=1, name='qT')
            for mh in range(8):
                if mh % 2 == 0:
                    q_ps = ppq.tile([128, 512], dt.float32, tag='qps', bufs=1,
                                    name='q_ps')
                else:
                    q_ps = pst.tile([128, 512], dt.float32, tag='sT', bufs=2,
                                    name='q_ps2')
                for i in range(2):
                    mc = mh * 2 + i
                    mm(q_ps[:, i * 256:(i + 1) * 256],
                       wq_r[:, :, mc * 128:(mc + 1) * 128], loc_r,
                       start=True, stop=True, perf_mode=DR)
                V.tensor_scalar_mul(out=qT[:, mh * 512:(mh + 1) * 512],
                                    in0=q_ps[:], scalar1=1.0 / 512.0)

            # --- weight loads for this layer (overlap downstream compute) ---
            wtc = sb.tile([128, 2048], dt.float8e4, tag='wtc', bufs=2,
                          name='wtc')
            nc.sync.dma_start(out=wtc[:], in_=P['WTC'][d])
            wdt = []
            for kcp in range(8):
                w = wdp.tile([128, 4096], dt.float8e4, tag='wd',
                             name=f'wd{kcp}')
                nc.sync.dma_start(out=w[:], in_=P['WDT'][d, kcp])
                wdt.append(w)
            wo = []
            for kcp in range(8):
                w = wdp.tile([128, 512], dt.float8e4, tag='wo',
                             name=f'wo{kcp}')
                nc.sync.dma_start(out=w[:], in_=P['WO'][d, kcp])
                wo.append(w)

            # --- C: rolled windows via shifted copies + boundary blend ---
            rolled = {}
            for nm, off in (('p1', -1), ('m1', +1)):
                rt = sb.tile([128, 512], dt.float8e4, tag=f'r{nm}',
                             name=f'r{nm}')
                rt_r = rt[:].rearrange('p (e j) -> p e j', e=2)
                if off == -1:
                    V.tensor_copy(out=rt_r[:, :, 1:256], in_=loc_r[:, :, 0:255])
                    # col 0 = partner row: local j=255 of the other rank
                    cand = (255, 511)   # (partner=rank0, partner=rank1)
                else:
                    V.tensor_copy(out=rt_r[:, :, 0:255], in_=loc_r[:, :, 1:256])
                    cand = (0, 256)     # m1: partner's j=0 -> (rank0: 0, rank1: 256)
                for ec in range(2):
                    c0, c1 = cand
                    ta = rows.tile([128, 1], dt.float32, tag='bta', bufs=4,
                                   name='bta')
                    tb = rows.tile([128, 1], dt.float32, tag='btb', bufs=4,
                                   name='btb')
                    V.tensor_mul(ta[:], full_bf[:, ec * 512 + c0: ec * 512 + c0 + 1],
                                 pmask[:, 0:1])
                    V.tensor_mul(tb[:], full_bf[:, ec * 512 + c1: ec * 512 + c1 + 1],
                                 pmask[:, 1:2])
                    dst = rt_r[:, ec, 0:1] if off == -1 else rt_r[:, ec, 255:256]
                    V.tensor_add(dst, ta[:], tb[:])
                rolled[nm] = rt

            # --- D: nat layout (rows on partitions) ---
            nat = nat_transpose(full_bf)
            nat_r = nat[:].rearrange('p (kb x) -> p kb x', kb=4)

            # --- E: local transition terms accumulated into xsad psum ---
            xsad_ps = ppx.tile([128, 512], dt.float32, tag='xsad',
                               name='xsad_ps')
            wtc_r = wtc[:].rearrange('p (mat k x) -> p mat k x', mat=4, k=2)

            def wtc_pair(mat, mc):
                return wtc_r[:, mat, :, mc * 128:(mc + 1) * 128]

            a1 = sb.tile([128, 512], dt.float8e4, tag='a1', name='a1')
            rp1_r = rolled['p1'][:].rearrange('p (e j) -> p e j', e=2)
            rm1_r = rolled['m1'][:].rearrange('p (e j) -> p e j', e=2)
            for mc in range(2):
                a_ps = ppy.tile([128, 256], dt.float32, tag='aps', name='a_ps')
                mm(a_ps[:], wtc_pair(0, mc), rp1_r, start=True, stop=True,
                   perf_mode=DR)
                act(out=a1[:, mc * 256:(mc + 1) * 256], in_=a_ps[:],
                    func=AF.Relu, scale=1.0 / 512.0)
            a1_r = a1[:].rearrange('p (e j) -> p e j', e=2)
            for mc in range(2):
                mm(xsad_ps[:, mc * 256:(mc + 1) * 256], wtc_pair(1, mc), a1_r,
                   start=True, stop=False, perf_mode=DR)
            a2 = sb.tile([128, 512], dt.float8e4, tag='a2', name='a2')
            for mc in range(2):
                a_ps = ppy.tile([128, 256], dt.float32, tag='aps',
                                name='a_ps2')
                mm(a_ps[:], wtc_pair(2, mc), rm1_r, start=True, stop=True,
                   perf_mode=DR)
                act(out=a2[:, mc * 256:(mc + 1) * 256], in_=a_ps[:],
                    func=AF.Relu, scale=1.0 / 512.0)
            a2_r = a2[:].rearrange('p (e j) -> p e j', e=2)
            for mc in range(2):
                mm(xsad_ps[:, mc * 256:(mc + 1) * 256], wtc_pair(3, mc), a2_r,
                   start=False, stop=False, perf_mode=DR)

            if debug and d == 0:
                t = dbg_out('dbg_rp1', [128, 512], dt.float8e4)
                nc.sync.dma_start(out=t[:], in_=rolled['p1'][:])
                t = dbg_out('dbg_rm1', [128, 512], dt.float8e4)
                nc.sync.dma_start(out=t[:], in_=rolled['m1'][:])
                t = dbg_out('dbg_a1', [128, 512], dt.float8e4)
                nc.sync.dma_start(out=t[:], in_=a1[:])
                t = dbg_out('dbg_q', [128, 4096], dt.float8e4)
                nc.sync.dma_start(out=t[:], in_=qT[:])

            # --- F: attention heads, transposed scores ---
            xid = sb.tile([128, 4096], dt.float8e4, tag='xid', bufs=1,
                          name='xid')
            xid_r = xid[:].rearrange('p (k j) -> p k j', k=16)
            qT_r = qT[:].rearrange('p (h k q) -> p h k q', h=8, k=2)

            def head_front(h):
                estT = sb.tile([128, 1024], dt.bfloat16, tag='estT', bufs=3,
                               name='estT')
                ssum = rows.tile([128, 2], dt.float32, tag='ssum', bufs=3,
                                 name='ssum')
                for qb in range(2):
                    sT = pst.tile([128, 512], dt.float32, tag='sT', bufs=2,
                                  name='sT')
                    mm(sT[:], qT_r[:, h, :, qb * 128:(qb + 1) * 128], full_r,
                       start=True, stop=True, perf_mode=DR)
                    act(out=estT[:, qb * 512:(qb + 1) * 512],
                        in_=sT[:], func=AF.Exp,
                        scale=1.0 / (16.0 * 65536.0))
                    V.reduce_sum(out=ssum[:, qb:qb + 1],
                                 in_=estT[:, qb * 512:(qb + 1) * 512],
                                 axis=AX.X)
                rec = rows.tile([128, 2], dt.float32, tag='rec', bufs=3,
                                name='rec')
                V.reciprocal(rec[:], ssum[:])
                dg = [rows.tile([128, 128], dt.bfloat16, tag=f'dg{qb}',
                                bufs=3, name=f'dg{qb}') for qb in range(2)]
                for qb in range(2):
                    nc.vector.tensor_scalar(
                        out=dg[qb][:], in0=ident[:], scalar1=rec[:, qb:qb + 1],
                        scalar2=SEST, op0=STT.mult, op1=STT.mult)
                return estT, dg

            def head_back(h, estT, dg):
                est = sb.tile([128, 1024], dt.float8e4, tag='est', bufs=3,
                              name='est')
                for kb in range(4):
                    tp = ppy.tile([128, 256], dt.float32, tag='aps',
                                  name='tpe')
                    for qb in range(2):
                        mm(tp[:, qb * 128:(qb + 1) * 128],
                           estT[:, qb * 512 + kb * 128: qb * 512 + kb * 128 + 128],
                           dg[qb][:], start=True, stop=True)
                    dst = est[:, kb * 256:(kb + 1) * 256]
                    if kb % 2 == 0:
                        V.tensor_copy(out=dst, in_=tp[:])
                    else:
                        act(out=dst, in_=tp[:], func=AF.Copy)
                est_r = est[:].rearrange('p (kb q) -> p kb q', kb=4)
                for ec in range(2):
                    y_ps = ppy.tile([128, 256], dt.float32, tag='aps',
                                    name='y_ps')
                    for kbp in range(2):
                        mm(y_ps[:],
                           nat_r[:, 2 * kbp:2 * kbp + 2,
                                 ec * 128:(ec + 1) * 128],
                           est_r[:, 2 * kbp:2 * kbp + 2, :],
                           start=(kbp == 0), stop=(kbp == 1), perf_mode=DR)
                    dst_x = xid[:, (h * 2 + ec) * 256:(h * 2 + ec + 1) * 256]
                    if h % 2 == 0:
                        act(out=dst_x, in_=y_ps[:], func=AF.Copy,
                            scale=1.0 / 1024.0)
                    else:
                        V.tensor_scalar_mul(out=dst_x, in0=y_ps[:],
                                            scalar1=1.0 / 1024.0)

            prev = None
            for h in range(8):
                cur = head_front(h)
                if prev is not None:
                    head_back(h - 1, *prev)
                if debug and d == 0 and h == 0:
                    t = dbg_out('dbg_estT0', [128, 1024], dt.bfloat16)
                    nc.sync.dma_start(out=t[:], in_=cur[0][:])
                prev = cur
            head_back(7, *prev)

            if debug and d == 0:
                t = dbg_out('dbg_xid', [128, 4096], dt.float8e4)
                nc.sync.dma_start(out=t[:], in_=xid[:])

            # --- G: dense relu (Wd), fp8 DoubleRow ---
            actb = sb.tile([128, 4096], dt.float8e4, tag='actb', bufs=1,
                           name='actb')
            for mc in range(16):
                act_ps = ppy.tile([128, 256], dt.float32, tag='aps',
                                  name='act_ps')
                for kcp in range(8):
                    wdr = wdt[kcp][:].rearrange('p (i x) -> p i x', i=2)
                    mm(act_ps[:], wdr[:, :, mc * 128:(mc + 1) * 128],
                       xid_r[:, 2 * kcp:2 * kcp + 2, :],
                       start=(kcp == 0), stop=(kcp == 7), perf_mode=DR)
                if mc % 2 == 0:
                    act(out=actb[:, mc * 256:(mc + 1) * 256], in_=act_ps[:],
                        func=AF.Relu, scale=1.0 / 1024.0)
                else:
                    V.tensor_scalar(out=actb[:, mc * 256:(mc + 1) * 256],
                                    in0=act_ps[:], scalar1=1.0 / 1024.0,
                                    scalar2=0.0, op0=STT.mult, op1=STT.max)
            if debug and d == 0:
                t = dbg_out('dbg_actb', [128, 4096], dt.float8e4)
                nc.sync.dma_start(out=t[:], in_=actb[:])

            # --- H: Wo accumulate into xsad ---
            actb_r = actb[:].rearrange('p (k j) -> p k j', k=16)
            for kcp in range(8):
                wor = wo[kcp][:].rearrange('p (i x) -> p i x', i=2)
                for mc in range(2):
                    mm(xsad_ps[:, mc * 256:(mc + 1) * 256],
                       wor[:, :, mc * 128:(mc + 1) * 128],
                       actb_r[:, 2 * kcp:2 * kcp + 2, :],
                       start=False, stop=(kcp == 7), perf_mode=DR)

            # --- I: norm, partition-major stats ---
            # u true = xsad_ps / 2^17
            u_bf = sb.tile([128, 512], dt.bfloat16, tag='u_bf', bufs=1,
                           name='u_bf')
            act(out=u_bf[:], in_=xsad_ps[:], func=AF.Copy,
                scale=1.0 / 131072.0)
            u2 = sb.tile([128, 512], dt.bfloat16, tag='u2', bufs=1, name='u2')
            V.tensor_mul(u2[:], u_bf[:], u_bf[:])
            xu = sb.tile([128, 512], dt.bfloat16, tag='xu', bufs=1, name='xu')
            V.tensor_mul(xu[:], master[:], u_bf[:])
            # stats psum (128, 6): [su0 su1 qu0 qu1 c0 c1]
            nrm_ps = pps.tile([128, 512], dt.float32, tag='nrm', bufs=1,
                              name='nrm_ps')
            stp = nrm_ps[:, 0:6]
            for si, s in enumerate((u_bf, u2, xu)):
                for jb in range(2):
                    for ec in range(2):
                        mm(stp[:, si * 2 + jb: si * 2 + jb + 1],
                           s[:, ec * 256 + jb * 128: ec * 256 + jb * 128 + 128],
                           ones_cb[:], start=(ec == 0), stop=(ec == 1))


            def row(nm):
                return rows.tile([128, 2], dt.float32, tag='rw', bufs=24,
                                 name=nm)

            stq = rows.tile([128, 6], dt.float32, tag='stq', bufs=2,
                            name='stq')
            V.tensor_copy(out=stq[:], in_=stp)
            su, qu, cc = stq[:, 0:2], stq[:, 2:4], stq[:, 4:6]
            bc_ps = nrm_ps
            t3, t5 = row('t3'), row('t5')
            V.scalar_tensor_tensor(out=t3[:], in0=su, scalar=-1.0 / E,
                                   in1=su, op0=STT.mult, op1=STT.mult)
            V.tensor_add(t5[:], t3[:], qu)
            stdu = row('stdu')
            act(out=stdu[:], in_=t5[:], func=AF.Sqrt, scale=1.0 / (E - 1))
            s1p, s1, alpha = row('s1p'), row('s1'), row('alpha')
            V.tensor_scalar_add(out=s1p[:], in0=stdu[:], scalar1=1.0)
            V.reciprocal(s1[:], s1p[:])
            V.tensor_scalar_mul(out=alpha[:], in0=s1[:], scalar1=STEP)
            asu, sy = row('asu'), row('sy')
            V.tensor_mul(asu[:], alpha[:], su)
            V.tensor_add(sy[:], asu[:], st_carry[:, 0:2])
            ac2, aa, aqu, qy0, qy = (row('ac2'), row('aa'), row('aqu'),
                                     row('qy0'), row('qy'))
            V.scalar_tensor_tensor(out=ac2[:], in0=alpha[:], scalar=2.0,
                                   in1=cc, op0=STT.mult, op1=STT.mult)
            V.tensor_mul(aa[:], alpha[:], alpha[:])
            V.tensor_mul(aqu[:], aa[:], qu)
            V.tensor_add(qy0[:], ac2[:], st_carry[:, 2:4])
            V.tensor_add(qy[:], qy0[:], aqu[:])
            t4, t5b = row('t4'), row('t5b')
            V.scalar_tensor_tensor(out=t4[:], in0=sy[:], scalar=-1.0 / E,
                                   in1=sy[:], op0=STT.mult, op1=STT.mult)
            V.tensor_add(t5b[:], t4[:], qy[:])
            stdy = row('stdy')
            act(out=stdy[:], in_=t5b[:], func=AF.Sqrt, scale=1.0 / (E - 1))
            s2p, s2, as2 = row('s2p'), row('s2'), row('as2')
            V.tensor_scalar_add(out=s2p[:], in0=stdy[:], scalar1=1.0)
            V.reciprocal(s2[:], s2p[:])
            V.tensor_mul(as2[:], alpha[:], s2[:])
            # carried stats for next layer (off critical path)
            st_carry = mpool.tile([128, 4], dt.float32, tag='stc', name='stc')
            s2sq = row('s2sq')
            V.tensor_mul(st_carry[:, 0:2], sy[:], s2[:])
            V.tensor_mul(s2sq[:], s2[:], s2[:])
            V.tensor_mul(st_carry[:, 2:4], qy[:], s2sq[:])
            # broadcast s2 / as2 over partitions via diag matmul (reuses the
            # nrm bank; the tile framework serializes it after the stq read)
            for vi, vv in enumerate((s2, as2)):
                for jb in range(2):
                    dgn = rows.tile([128, 128], dt.bfloat16, tag='dgn',
                                    bufs=4, name='dgn')
                    V.tensor_scalar_mul(out=dgn[:], in0=ident[:],
                                        scalar1=vv[:, jb:jb + 1])
                    mm(bc_ps[:, vi * 256 + jb * 128: vi * 256 + jb * 128 + 128],
                       ones128[:], dgn[:], start=True, stop=True)
            newmaster = mpool.tile([128, 512], dt.float32, tag='master',
                                   name='master')
            for ec in range(2):
                ta = sb.tile([128, 256], dt.float32, tag='tmp', bufs=2,
                             name='ta')
                V.tensor_mul(ta[:], u_bf[:, ec * 256:(ec + 1) * 256],
                             bc_ps[:, 256:512])
                V.tensor_mul(newmaster[:, ec * 256:(ec + 1) * 256],
                             master[:, ec * 256:(ec + 1) * 256],
                             bc_ps[:, 0:256])
                V.tensor_add(newmaster[:, ec * 256:(ec + 1) * 256],
                             newmaster[:, ec * 256:(ec + 1) * 256], ta[:])
            master = newmaster
            loc_bf = mpool.tile([128, 512], dt.bfloat16, tag='locb',
                                name='locb')
            act(out=loc_bf[:], in_=master[:], func=AF.Copy)
            loc = mpool.tile([128, 512], dt.float8e4, tag='loc', name='loc')
            V.tensor_scalar_mul(out=loc[:], in0=master[:], scalar1=SA)
            if debug:
                t = dbg_out(f'dbg_xsa{d}', [128, 512], dt.float32)
                nc.sync.dma_start(out=t[:], in_=master[:])

        # ================= HEAD =================
        full_bf, full = comm_gather(loc_bf)
        full_r = full[:].rearrange('p (e g) -> p e g', e=2)
        nat = nat_transpose(full_bf)

        # lptok: (e, j) per pair batch; fp8 x256
        lptok = hp.tile([128, 128], dt.float8e4, name='lptok')
        for ec in range(2):
            l_ps = ppy.tile([128, 64], dt.float32, tag='aps', name='l_ps')
            for lb in range(4):
                mm(l_ps[:], nat[:, lb * 256 + ec * 128: lb * 256 + ec * 128 + 128],
                   msel[:, lb * 64:(lb + 1) * 64],
                   start=(lb == 0), stop=(lb == 3))
            V.tensor_copy(out=lptok[:, ec * 64:(ec + 1) * 64], in_=l_ps[:])

        # xx: kchoice (e, n) n = j*4+kn; psum = 2^17 true; out fp8 x256
        xxsb = hp.tile([128, 512], dt.float8e4, name='xxsb')
        for kn in range(KN):
            for ec in range(2):
                x_ps = ppy.tile([128, 64], dt.float32, tag='aps', name='x_ps')
                for fc in range(2):
                    off = (fc * 8 + kn * 2 + ec) * 128
                    mm(x_ps[:], wkct[:, off:off + 128],
                       lptok[:, fc * 64:(fc + 1) * 64],
                       start=(fc == 0), stop=(fc == 1))
                dst = xxsb[:, ec * 256:(ec + 1) * 256].rearrange(
                    'p (j f) -> p f j', f=4)[:, kn, :]
                nc.vector.tensor_scalar(
                    out=dst, in0=x_ps[:],
                    scalar1=bkc_sb[:, kn * 2 + ec: kn * 2 + ec + 1],
                    scalar2=1.0 / 512.0, op0=STT.add, op1=STT.mult)

        # xx2T: (l, n) blocks; fp8 x256
        xxsb_r = xxsb[:].rearrange('p (e n) -> p e n', e=2)
        xx2 = hp.tile([128, 1024], dt.float8e4, name='xx2')
        for lb in range(4):
            x_ps = ppy.tile([128, 256], dt.float32, tag='aps', name='x2_ps')
            mm(x_ps[:], full_r[:, :, lb * 128:(lb + 1) * 128], xxsb_r,
               start=True, stop=True, perf_mode=DR)
            V.tensor_scalar_mul(out=xx2[:, lb * 256:(lb + 1) * 256],
                                in0=x_ps[:], scalar1=1.0 / 256.0)

        # xx3T: (e, n); fp8 x256
        xx2_r = xx2[:].rearrange('p (kb n) -> p kb n', kb=4)
        nath_r = nat[:].rearrange('p (kb x) -> p kb x', kb=4)
        xx3 = hp.tile([128, 512], dt.float8e4, name='xx3')
        for ec in range(2):
            x_ps = ppy.tile([128, 256], dt.float32, tag='aps', name='x3_ps')
            for kbp in range(2):
                mm(x_ps[:],
                   nath_r[:, 2 * kbp:2 * kbp + 2, ec * 128:(ec + 1) * 128],
                   xx2_r[:, 2 * kbp:2 * kbp + 2, :],
                   start=(kbp == 0), stop=(kbp == 1), perf_mode=DR)
            V.tensor_scalar_mul(out=xx3[:, ec * 256:(ec + 1) * 256],
                                in0=x_ps[:], scalar1=1.0 / 256.0)

        # xxWT: (e, n); fp8 x256
        wem_r = wem[:].rearrange('p (k x) -> p k x', k=2)
        xx3_r = xx3[:].rearrange('p (k n) -> p k n', k=2)
        xxw = hp.tile([128, 512], dt.float8e4, name='xxw')
        for ec in range(2):
            x_ps = ppy.tile([128, 256], dt.float32, tag='aps', name='xw_ps')
            mm(x_ps[:], wem_r[:, :, ec * 128:(ec + 1) * 128], xx3_r,
               start=True, stop=True, perf_mode=DR)
            V.tensor_scalar_mul(out=xxw[:, ec * 256:(ec + 1) * 256],
                                in0=x_ps[:], scalar1=1.0 / 512.0)

        # all-gather xxW across batches (fp8)
        xxw_in = dram.tile([2, 128, 256], dt.float8e4, tag='xxw_in')
        xxw_out = dram.tile([8, 128, 256], dt.float8e4, tag='xxw_out')
        for ec in range(2):
            nc.sync.dma_start(out=xxw_in[ec], in_=xxw[:, ec * 256:(ec + 1) * 256])
        nc.gpsimd.collective_compute(
            "AllGather", mybir.AluOpType.bypass,
            replica_groups=[[0, 2, 4, 6], [1, 3, 5, 7]],
            ins=[xxw_in.opt()], outs=[xxw_out.opt()],
        )
        # xxwall: (128, 2, 1024) fp8: [p, ec, n] n = b*256 + j*4 + kn
        xxwall = hp.tile([128, 2048], dt.float8e4, name='xxwall')
        xxwall_r = xxwall[:].rearrange('p (e n) -> p e n', e=2)
        for bb in range(4):
            for ec in range(2):
                nc.sync.dma_start(
                    out=xxwall_r[:, ec, bb * 256:(bb + 1) * 256],
                    in_=xxw_out[bb * 2 + ec])
        if debug:
            t = dbg_out('dbg_lptok', [128, 128], dt.float8e4)
            nc.sync.dma_start(out=t[:], in_=lptok[:])
            t = dbg_out('dbg_xx', [128, 512], dt.float8e4)
            nc.sync.dma_start(out=t[:], in_=xxsb[:])
            t = dbg_out('dbg_xx2', [128, 1024], dt.float8e4)
            nc.sync.dma_start(out=t[:], in_=xx2[:])
            t = dbg_out('dbg_xx3', [128, 512], dt.float8e4)
            nc.sync.dma_start(out=t[:], in_=xx3[:])
            t = dbg_out('dbg_xxwall', [128, 2048], dt.float8e4)
            nc.sync.dma_start(out=t[:], in_=xxwall[:])

        # clog: per-row dot of xxW with target embedding (tb = 2^17 true)
        tb = hp.tile([128, 2048], dt.bfloat16, name='tb')
        ett_r = ett[:].rearrange('p (e n) -> p e n', e=2)
        for ec in range(2):
            V.tensor_mul(tb[:].rearrange('p (e n) -> p e n', e=2)[:, ec, :],
                         xxwall_r[:, ec, :], ett_r[:, ec, :])
        clog_d = dram.tile([1024], dt.float32, tag='clog_d')
        tb_r = tb[:].rearrange('p (e n) -> p e n', e=2)
        for half in range(2):
            cl_t = pps.tile([128, 512], dt.float32, tag='nrm', bufs=1,
                            name='cl_t')
            cl_ps = cl_t[0:1, :]
            for ec in range(2):
                mm(cl_ps, ones_cb[:],
                   tb_r[:, ec, half * 512:(half + 1) * 512],
                   start=(ec == 0), stop=(ec == 1))
            cl_sb = hp.tile([1, 512], dt.float32, tag='cl_sb', bufs=2,
                            name='cl_sb')
            act(out=cl_sb[:], in_=cl_ps[:], func=AF.Copy,
                scale=1.0 / 131072.0)
            nc.sync.dma_start(out=clog_d[half * 512:(half + 1) * 512],
                              in_=cl_sb[:])
        if debug:
            t = dbg_out('dbg_clog', [1024], dt.float32)
            nc.sync.dma_start(out=t[:], in_=clog_d[:])

        # logits + per-shard sum-exp (vocab parallel, fp8 DoubleRow),
        # two row-half passes; each half's stats AllGather overlaps the rest
        st_outs = []
        for half in range(2):
            stats = hp.tile([128, 32], dt.float32, tag='hstats', bufs=2,
                            name='stats')
            for vc in range(NVC):
                er = embt[vc][:].rearrange('p (e n) -> p e n', e=2)
                for nbh in range(4):
                    nb = half * 4 + nbh
                    if nbh % 2 == 0:
                        lg_ps = ppq.tile([128, VC], dt.float32, tag='qps',
                                         name='lg_ps')
                    else:
                        lg_ps = pst.tile([128, VC], dt.float32, tag='sT',
                                         bufs=2, name='lg_ps2')
                    mm(lg_ps[:], xxwall_r[:, :, nb * 128:(nb + 1) * 128], er,
                       start=True, stop=True, perf_mode=DR)
                    escr = hp.tile([128, VC], dt.bfloat16, tag='escr', bufs=2,
                                   name='escr')
                    act(out=escr[:], in_=lg_ps[:], func=AF.Exp,
                        scale=1.0 / 131072.0,
                        accum_out=stats[:, nbh * 8 + vc: nbh * 8 + vc + 1])
            se = hp.tile([128, 4], dt.float32, tag='se', bufs=2, name='se')
            for nbh in range(4):
                V.reduce_sum(out=se[:, nbh:nbh + 1],
                             in_=stats[:, nbh * 8:(nbh + 1) * 8], axis=AX.X)
            st_in = dram.tile([512], dt.float32, tag='st_in')
            st_out = dram.tile([512], dt.float32, tag='st_out',
                               addr_space="Shared")
            nc.gpsimd.dma_start(
                out=st_in[:].rearrange('(nb p) -> p nb', p=128), in_=se[:])
            nc.gpsimd.collective_compute(
                "AllReduce", mybir.AluOpType.add,
                replica_groups=[list(range(NCORES))],
                ins=[st_in.opt()], outs=[st_out.opt()],
            )
            st_outs.append(st_out)

        # combine gathered partial sum-exps + cent + weighted sum
        cent = hp.tile([128, 2], dt.float32, name='cent')
        lse_g = hp.tile([128, 8], dt.float32, tag='lse_g', name='lse_g')
        cg = hp.tile([128, 8], dt.float32, tag='cg', name='cg')
        for t_ in range(2):
            nc.sync.dma_start(
                out=lse_g[:, t_ * 4:(t_ + 1) * 4],
                in_=st_outs[t_][:].rearrange('(p f) -> p f', f=4))
            nc.sync.dma_start(
                out=cg[:, t_ * 4:(t_ + 1) * 4],
                in_=clog_d[t_ * 512:(t_ + 1) * 512].rearrange(
                    '(p f) -> p f', f=4))
        lse = hp.tile([128, 8], dt.float32, tag='lse', name='lse')
        act(out=lse[:], in_=lse_g[:], func=AF.Ln)
        df = hp.tile([128, 8], dt.float32, tag='df', name='df')
        V.tensor_sub(df[:], cg[:], lse[:])
        ex = hp.tile([128, 8], dt.float32, tag='ex', name='ex')
        act(out=ex[:], in_=df[:], func=AF.Exp)
        for t_ in range(2):
            sm = hp.tile([128, 1], dt.float32, tag='sm', bufs=2, name='sm')
            V.reduce_sum(out=sm[:], in_=ex[:, t_ * 4:(t_ + 1) * 4], axis=AX.X)
            act(out=cent[:, t_:t_ + 1], in_=sm[:], func=AF.Ln)
        num_t = pps.tile([128, 512], dt.float32, tag='nrm', bufs=1,
                         name='num_t')
        num_ps = num_t[0:4, 0:1]
        for t_ in range(2):
            mm(num_ps, wind[:, t_ * 4:(t_ + 1) * 4], cent[:, t_:t_ + 1],
               start=(t_ == 0), stop=(t_ == 1))
        outsb = hp.tile([4, 1], dt.float32, name='outsb')
        V.tensor_copy(out=outsb[:], in_=num_ps)
        nc.sync.dma_start(out=out_t[:], in_=outsb[:])
        if debug:
            t = dbg_out('dbg_cent', [128, 2], dt.float32)
            nc.sync.dma_start(out=t[:], in_=cent[:])

    nc.compile()
    return nc


def kernel(**inputs):
    from concourse.bass_utils import run_bass_kernel_spmd

    in_maps, aux = _prep(inputs)
    key = (_D_EFF, _DEBUG)
    if key not in _CACHE:
        _CACHE[key] = _build(_D_EFF, _DEBUG)
    nc = _CACHE[key]
    res = run_bass_kernel_spmd(nc, in_maps, list(range(NCORES)), trace=_TRACE)
    kernel._last_results = res
    num = np.asarray(res.results[0]['out'], np.float32)[:, 0]
    summer = aux['summer']
    sumw = summer.sum(-1)
    loss = -(num - np.log(KN) * sumw) / np.clip(sumw, 1.0, None)
    return loss.astype(np.float32)
